# revision 1
# baseline (speedup 1.0000x reference)
"""Global-attention kernel for [8, 384, 32, 32] ConvAttention on 8 trn2 cores.

Math (per reference): tokens over B*H*W = 8192 positions, C = 384 channels
split as V/K/Q of 128 each; out = softmax(Q K^T / sqrt(128)) V, re-laid as
[B, 128, H, W].

Sharding: core c owns the 1024 query tokens of batch c (token n = b*1024+hw,
so batch == contiguous token block). K/V are replicated. Each core computes
its row block of the attention entirely locally; no collectives.

On-core layout: channel-major ([d, token]) everywhere, S^T formulation:
for each kv chunk j (128 tokens), S^T_j = K_j^T Q in PSUM (3 rotating
tiles), exp on ACT (or a Schraudolph bit-trick exp on DVE for a minority of
chunks, splitting the elementwise load across both engines; those are
emitted right after their QK so they overlap ACT's exp of earlier chunks),
then O^T += V_j^T E_j on PE.

All e-side tensors are bf16 (logits reach ~|21| after scaling, so exp spans
e^-21..e^21 — beyond fp16 range). Softmax-denominator partials accumulate
as one bf16 tensor_add per chunk on DVE (2x perf mode) into a [128, 2048]
accumulator split by chunk parity; each parity region is folded (with
partition broadcast) by ones-matrix matmuls as soon as its last chunk is
summed, and the final chunk's e folds directly so the tail chain is short.
The last chunk's exp is split ACT/DVE by query half for the same reason.
Normalize = reciprocal + multiply per query half.
"""

import math

import numpy as np

import concourse.bass as bass
import concourse.tile as tile
from concourse import bacc, mybir
from concourse.alu_op_type import AluOpType
from concourse.bass_utils import run_bass_kernel_spmd

N_CORES = 8
B, C, H, W = 8, 384, 32, 32
HW = H * W            # 1024 tokens per batch == per core
N = B * HW            # 8192 total tokens
D = 128               # key/value width
NCHUNK = N // 128     # 64 kv chunks of 128 tokens
SCALE = 1.0 / math.sqrt(D)
F32 = mybir.dt.float32
F32R = mybir.dt.float32r
F16 = mybir.dt.float16
BF16 = mybir.dt.bfloat16
I16 = mybir.dt.int16

# Schraudolph exp on DVE (bf16 bit trick): i16 = x*a + b truncated to int16;
# the bit pattern read as bf16 approximates exp(x*SCALE) within ~3.5%.
A7S = float((1 << 7) / math.log(2.0) * SCALE)
B7 = float(127 * (1 << 7) - 6)

# Chunks whose exp runs on DVE via the bit trick (the rest exp on ACT).
# Kept clear of the final chunks, which sit on the drain-critical path.
DVE_CHUNKS = frozenset(c for c in range(NCHUNK - 2) if c % 6 == 2)  # 10

N_WARMUP_MM = 14  # PE p-state warmup matmuls issued while input DMAs land


def _build_nc():
    nc = bacc.Bacc(
        "TRN2", target_bir_lowering=False, debug=False, num_devices=N_CORES
    )
    qT = nc.dram_tensor("qT", [D, HW], F32, kind="ExternalInput").ap()
    kT00 = nc.dram_tensor("kT00", [D, D], F32, kind="ExternalInput").ap()
    kT = nc.dram_tensor("kT", [D, N], F32, kind="ExternalInput").ap()
    vt16 = nc.dram_tensor("vt16", [D, N], F16, kind="ExternalInput").ap()
    oT = nc.dram_tensor("oT", [D, HW], F32, kind="ExternalOutput").ap()

    with tile.TileContext(nc) as tc:
        with (
            tc.tile_pool(name="persist", bufs=1) as persist,
            tc.tile_pool(name="etile", bufs=8) as epool,
            tc.tile_pool(name="spsum", bufs=3, space="PSUM") as spsum,
            tc.tile_pool(name="apsum", bufs=1, space="PSUM") as apsum,
        ):
            # --- SBUF persistents ---
            qT_sb = persist.tile([D, HW], F32R, tag="qT_sb")
            kT00_sb = persist.tile([D, D], F32R, tag="kT00_sb")
            kT_sb = [
                persist.tile([D, HW], F32R, tag=f"kT{i}", name=f"kT_sb{i}")
                for i in range(8)
            ]
            vt_sb = [
                persist.tile([D, HW], F16, tag=f"vt{i}", name=f"vt_sb{i}")
                for i in range(8)
            ]
            ones16 = persist.tile([D, D], BF16, tag="ones16")
            # Denominator partials, split by chunk parity (region 0: even
            # chunks, region 1: odd) so each region folds early.
            rs2 = persist.tile([D, 2 * HW], BF16, tag="rs2")
            warm_sb = persist.tile([D, 256], F32, tag="warm_sb")

            # Cheap init first so PE warmup starts immediately; rs2 needs no
            # init (first touch of each region is a copy).
            scr32 = persist.tile([D, D], F32, tag="scr32")
            nc.gpsimd.memset(warm_sb[:], 0.5)
            nc.gpsimd.memset(scr32[:], 1.0)
            nc.gpsimd.tensor_copy(ones16[:], scr32[:])

            # --- input DMAs, latency-critical pieces first ---
            nc.sync.dma_start(out=kT00_sb[:], in_=kT00[:].bitcast(F32R))
            nc.sync.dma_start(out=qT_sb[:, 0:512], in_=qT[:, 0:512].bitcast(F32R))
            nc.sync.dma_start(out=qT_sb[:, 512:1024], in_=qT[:, 512:1024].bitcast(F32R))
            for i in range(8):
                nc.sync.dma_start(
                    out=kT_sb[i][:], in_=kT[:, i * HW : (i + 1) * HW].bitcast(F32R)
                )
                nc.sync.dma_start(
                    out=vt_sb[i][:], in_=vt16[:, i * HW : (i + 1) * HW]
                )

            # --- PE warmup: keep the tensor engine busy (and ramping to
            # full clock) while the first input DMAs land; results unused.
            wm_ps = spsum.tile([D, HW], F32, tag="s", name="warm_ps")
            for i in range(N_WARMUP_MM):
                nc.tensor.matmul(
                    wm_ps[:, 0:256],
                    warm_sb[:, 0:128].bitcast(F32R),
                    warm_sb[:].bitcast(F32R),
                    start=True,
                    stop=True,
                )

            o_psum = apsum.tile([D, HW], F32, tag="o_psum")

            def kchunk(c):
                if c == 0:
                    return kT00_sb[:]
                blk, off = c // 8, (c % 8) * 128
                return kT_sb[blk][:, off : off + 128]

            def vchunk(c):
                blk, off = c // 8, (c % 8) * 128
                return vt_sb[blk][:, off : off + 128]

            def emit_qk(c):
                s_ps = spsum.tile([D, HW], F32, tag="s", name=f"s_ps{c}")
                for h in range(2):
                    nc.tensor.matmul(
                        s_ps[:, h * 512 : (h + 1) * 512],
                        kchunk(c),
                        qT_sb[:, h * 512 : (h + 1) * 512],
                        start=True,
                        stop=True,
                    )
                return s_ps

            def emit_exp_dve(c, s_ps, sl=slice(0, HW)):
                e_i16 = epool.tile([D, HW], I16, tag="e", name=f"e{c}")
                nc.vector.tensor_scalar(
                    out=e_i16[:, sl],
                    in0=s_ps[:, sl],
                    scalar1=A7S,
                    scalar2=B7,
                    op0=AluOpType.mult,
                    op1=AluOpType.add,
                )
                return e_i16

            # rs_bc_ps is allocated from the spsum pool near the end (the
            # rotation frees a slot exactly when the first fold runs).
            rs_bc_ps = None
            fold_state = {0: True, 1: True}  # per-q-half "is first matmul"

            def emit_fold(src, final):
                for h in range(2):
                    nc.tensor.matmul(
                        rs_bc_ps[:, h * 512 : (h + 1) * 512],
                        ones16[:],
                        src[:, h * 512 : (h + 1) * 512],
                        start=fold_state[h],
                        stop=final,
                    )
                    fold_state[h] = False

            # Software-pipelined two chunks ahead (3 PSUM S-slots). DVE-exp
            # chunks are emitted right after their QK.
            s_tiles = {0: emit_qk(0), 1: emit_qk(1)}
            e_early = {}
            for c in range(NCHUNK):
                if c + 2 < NCHUNK:
                    s_tiles[c + 2] = emit_qk(c + 2)
                    if c + 2 in DVE_CHUNKS:
                        e_early[c + 2] = emit_exp_dve(c + 2, s_tiles[c + 2])
                s_ps = s_tiles.pop(c)

                if c in DVE_CHUNKS:
                    e16 = e_early.pop(c)[:].bitcast(BF16)
                elif c == NCHUNK - 1:
                    # Last chunk: split the exp across ACT (half 0) and DVE
                    # (half 1, bit trick) to shorten the drain chain.
                    e_sb = epool.tile([D, HW], BF16, tag="e", name=f"e{c}")
                    nc.scalar.activation(
                        e_sb[:, 0:512],
                        s_ps[:, 0:512],
                        mybir.ActivationFunctionType.Exp,
                        scale=SCALE,
                    )
                    nc.vector.tensor_scalar(
                        out=e_sb[:, 512:1024].bitcast(I16),
                        in0=s_ps[:, 512:1024],
                        scalar1=A7S,
                        scalar2=B7,
                        op0=AluOpType.mult,
                        op1=AluOpType.add,
                    )
                    e16 = e_sb[:]
                else:
                    e_sb = epool.tile([D, HW], BF16, tag="e", name=f"e{c}")
                    nc.scalar.activation(
                        e_sb[:],
                        s_ps[:],
                        mybir.ActivationFunctionType.Exp,
                        scale=SCALE,
                    )
                    e16 = e_sb[:]

                for h in range(2):
                    nc.tensor.matmul(
                        o_psum[:, h * 512 : (h + 1) * 512],
                        vchunk(c),
                        e16[:, h * 512 : (h + 1) * 512],
                        start=(c == 0),
                        stop=(c == NCHUNK - 1),
                    )

                # Denominator partials (skip the last chunk: its e folds
                # directly on PE). First touch of each region is a copy.
                if c < NCHUNK - 1:
                    reg = (c % 2) * HW
                    region = rs2[:, reg : reg + HW]
                    if c < 2:
                        nc.vector.tensor_copy(region, e16)
                    else:
                        nc.vector.tensor_add(region, region, e16)

                if c == NCHUNK - 3:
                    # chunk 61 was the last odd rs2 contributor
                    rs_bc_ps = spsum.tile([D, HW], F32, tag="s", name="rs_bc_ps")
                    emit_fold(rs2[:, HW : 2 * HW], final=False)
                elif c == NCHUNK - 2:
                    # chunk 62 was the last even rs2 contributor
                    emit_fold(rs2[:, 0:HW], final=False)
                elif c == NCHUNK - 1:
                    emit_fold(e16, final=True)

            # --- endgame: rs_bc_ps holds the full denominator replicated
            # across partitions; normalize and store per query half.
            for h in range(2):
                sl = slice(h * 512, (h + 1) * 512)
                rec_sb = persist.tile([D, 512], F32, tag=f"rec{h}")
                nc.vector.reciprocal(rec_sb[:], rs_bc_ps[:, sl])
                o_sb = persist.tile([D, 512], F32, tag=f"osb{h}")
                nc.vector.tensor_tensor(
                    o_sb[:], o_psum[:, sl], rec_sb[:], AluOpType.mult
                )
                nc.sync.dma_start(out=oT[:, sl], in_=o_sb[:])

    nc.compile()
    return nc


_NC_CACHE = None


def _get_nc():
    global _NC_CACHE
    if _NC_CACHE is None:
        _NC_CACHE = _build_nc()
    return _NC_CACHE


def _prep_inputs(x: np.ndarray) -> list[dict]:
    x = np.ascontiguousarray(x, dtype=np.float32)
    xr = x.reshape(B, C, HW)

    # K channel-major over all tokens: kT[d, b*1024+hw] = x[b, 128+d, hw]
    kT = np.ascontiguousarray(xr[:, 128:256, :].transpose(1, 0, 2)).reshape(D, N)
    kT00 = np.ascontiguousarray(kT[:, 0:128])
    # V chunk-transposed fp16: vt[p, 128*j + v] = V[128*j + p, v]
    v_tok = np.ascontiguousarray(xr[:, 0:128, :].transpose(0, 2, 1)).reshape(N, D)
    vt16 = np.ascontiguousarray(
        v_tok.reshape(NCHUNK, 128, D).transpose(1, 0, 2)
    ).reshape(D, N).astype(np.float16)

    in_maps = []
    for c in range(N_CORES):
        qT = np.ascontiguousarray(xr[c, 256:384, :])
        in_maps.append({"qT": qT, "kT00": kT00, "kT": kT, "vt16": vt16})
    return in_maps


def kernel(x: np.ndarray) -> np.ndarray:
    assert x.shape == (B, C, H, W), x.shape
    in_maps = _prep_inputs(x)
    nc = _get_nc()
    res = run_bass_kernel_spmd(nc, in_maps, list(range(N_CORES)))

    out = np.empty((B, D, H, W), dtype=np.float32)
    for c in range(N_CORES):
        out[c] = res.results[c]["oT"].reshape(D, H, W)
    return out



# revision 7
# speedup vs baseline: 1.0506x; 1.0506x over previous
"""Global-attention kernel for [8, 384, 32, 32] ConvAttention on 8 trn2 cores.

Math (per reference): tokens over B*H*W = 8192 positions, C = 384 channels
split as V/K/Q of 128 each; out = softmax(Q K^T / sqrt(128)) V, re-laid as
[B, 128, H, W].

Sharding: core c owns the 1024 query tokens of batch c (token n = b*1024+hw,
so batch == contiguous token block). K/V are replicated. Each core computes
its row block of the attention entirely locally; no collectives.

On-core layout: channel-major ([d, token]) everywhere, S^T formulation:
for each kv chunk j (128 tokens), S^T_j = K_j^T Q in PSUM (3 rotating
tiles), exp to bf16, then O^T += V_j^T E_j on PE, accumulated in two
half-tiles (cols 0:512 / 512:1024) so the first half drains one matmul
early.

Q/K are bf16 (halves input DMA vs f32; verified ~1e-2 rel err), V fp16.
The PE stream is the bottleneck (QK + AV = 2048 rows/chunk at 1 row/cycle,
2.4 GHz); every 4th chunk's exp runs as a Schraudolph bit-trick on Pool
(mostly) or DVE so the ACT engine (1038ns/exp vs 854ns chunk period) never
falls more than ~550ns behind and the PE never stalls on an e-tile or
PSUM-slot rotation.

Softmax denominator: bf16 adds on DVE into parity regions of rs2; odd
region folds (ones-matmul, partition-broadcast) after chunk 61, even after
62, and chunk 63's e folds directly, all before the last two AV matmuls so
the reciprocal (DVE) starts 426ns before the PE drains. Normalize+store is
pipelined in 4 query-column slices across DVE/Pool with 4 output DMAs.
"""

import math

import numpy as np
import ml_dtypes

import concourse.bass as bass
import concourse.tile as tile
from concourse import bacc, mybir
from concourse.alu_op_type import AluOpType
from concourse.bass_utils import run_bass_kernel_spmd

N_CORES = 8
B, C, H, W = 8, 384, 32, 32
HW = H * W            # 1024 tokens per batch == per core
N = B * HW            # 8192 total tokens
D = 128               # key/value width
NCHUNK = N // 128     # 64 kv chunks of 128 tokens
SCALE = 1.0 / math.sqrt(D)
F32 = mybir.dt.float32
F32R = mybir.dt.float32r
F16 = mybir.dt.float16
BF16 = mybir.dt.bfloat16
I16 = mybir.dt.int16

# Schraudolph exp (bf16 bit trick): i16 = x*a + b truncated to int16; the
# bit pattern read as bf16 approximates exp(x*SCALE) within ~3.5%.
A7S = float((1 << 7) / math.log(2.0) * SCALE)
B7 = float(127 * (1 << 7) - 6)

# Every 4th chunk's exp runs off the ACT engine (period-4 pattern keeps the
# ACT backlog bounded). GPSIMD cannot read PSUM on TRN2, so all of these
# run on DVE; to compensate, ~1 in 5 denominator adds (SBUF-only) moves to
# Pool.
SCHR = frozenset(c for c in range(NCHUNK) if c % 4 == 1)  # 16 chunks
POOL_RS = frozenset(c for c in range(2, 55) if c % 5 == 2)  # 11 chunks

N_WARMUP_MM = 8  # PE matmuls filling the pstate-ramp window while DMAs land


def _build_nc():
    nc = bacc.Bacc(
        "TRN2", target_bir_lowering=False, debug=False, num_devices=N_CORES
    )
    qT = nc.dram_tensor("qT", [D, HW], BF16, kind="ExternalInput").ap()
    kT00 = nc.dram_tensor("kT00", [D, D], BF16, kind="ExternalInput").ap()
    kT = nc.dram_tensor("kT", [D, N], BF16, kind="ExternalInput").ap()
    vt16 = nc.dram_tensor("vt16", [D, N], F16, kind="ExternalInput").ap()
    oT = nc.dram_tensor("oT", [D, HW], F32, kind="ExternalOutput").ap()

    with tile.TileContext(nc) as tc:
        with (
            tc.tile_pool(name="persist", bufs=1) as persist,
            tc.tile_pool(name="etile", bufs=10) as epool,
            tc.tile_pool(name="spsum", bufs=3, space="PSUM") as spsum,
            tc.tile_pool(name="apsum", bufs=1, space="PSUM") as apsum,
        ):
            # --- SBUF persistents ---
            qT_sb = persist.tile([D, HW], BF16, tag="qT_sb")
            kT00_sb = persist.tile([D, D], BF16, tag="kT00_sb")
            kT_sb = [
                persist.tile([D, HW], BF16, tag=f"kT{i}", name=f"kT_sb{i}")
                for i in range(8)
            ]
            vt_sb = [
                persist.tile([D, HW], F16, tag=f"vt{i}", name=f"vt_sb{i}")
                for i in range(8)
            ]
            ones16 = persist.tile([D, D], BF16, tag="ones16")
            # Denominator partials: region 0 = even chunks, 1 = odd.
            rs2 = persist.tile([D, 2 * HW], BF16, tag="rs2")
            warm_sb = persist.tile([D, 256], F32, tag="warm_sb")

            nc.gpsimd.memset(warm_sb[:], 0.5)
            nc.gpsimd.memset(ones16[:], 1.0)

            # --- input DMAs, latency-critical pieces first ---
            nc.sync.dma_start(out=kT00_sb[:], in_=kT00[:])
            nc.sync.dma_start(out=qT_sb[:, 0:512], in_=qT[:, 0:512])
            nc.sync.dma_start(out=qT_sb[:, 512:1024], in_=qT[:, 512:1024])
            for i in range(8):
                nc.sync.dma_start(
                    out=kT_sb[i][:], in_=kT[:, i * HW : (i + 1) * HW]
                )
                nc.sync.dma_start(
                    out=vt_sb[i][:], in_=vt16[:, i * HW : (i + 1) * HW]
                )

            # --- PE warmup: occupy the sub-3us pstate window while the
            # first input DMAs land; results unused.
            wm_ps = spsum.tile([D, HW], F32, tag="s", name="warm_ps")
            for i in range(N_WARMUP_MM):
                nc.tensor.matmul(
                    wm_ps[:, 0:256],
                    warm_sb[:, 0:128].bitcast(F32R),
                    warm_sb[:].bitcast(F32R),
                    start=True,
                    stop=True,
                )

            # O^T accumulators, one PSUM bank per query half.
            o_ps = [
                apsum.tile([D, 512], F32, tag=f"o_ps{h}", name=f"o_ps{h}")
                for h in range(2)
            ]

            def kchunk(c):
                if c == 0:
                    return kT00_sb[:]
                blk, off = c // 8, (c % 8) * 128
                return kT_sb[blk][:, off : off + 128]

            def vchunk(c):
                blk, off = c // 8, (c % 8) * 128
                return vt_sb[blk][:, off : off + 128]

            def emit_qk(c):
                s_ps = spsum.tile([D, HW], F32, tag="s", name=f"s_ps{c}")
                for h in range(2):
                    nc.tensor.matmul(
                        s_ps[:, h * 512 : (h + 1) * 512],
                        kchunk(c),
                        qT_sb[:, h * 512 : (h + 1) * 512],
                        start=True,
                        stop=True,
                    )
                return s_ps

            # e-producer, emitted right after the chunk's QK so the engine
            # starts the moment the PSUM tile is ready.
            def emit_e(c, s_ps):
                if c in SCHR:
                    e_i16 = epool.tile([D, HW], I16, tag="e", name=f"e{c}")
                    nc.vector.tensor_scalar(
                        out=e_i16[:],
                        in0=s_ps[:],
                        scalar1=A7S,
                        scalar2=B7,
                        op0=AluOpType.mult,
                        op1=AluOpType.add,
                    )
                    return e_i16[:].bitcast(BF16)
                e_sb = epool.tile([D, HW], BF16, tag="e", name=f"e{c}")
                nc.scalar.activation(
                    e_sb[:],
                    s_ps[:],
                    mybir.ActivationFunctionType.Exp,
                    scale=SCALE,
                )
                return e_sb[:]

            rs_bc_ps = None
            fold_started = {0: False, 1: False}

            def emit_fold(src, final):
                for h in range(2):
                    nc.tensor.matmul(
                        rs_bc_ps[:, h * 512 : (h + 1) * 512],
                        ones16[:],
                        src[:, h * 512 : (h + 1) * 512],
                        start=not fold_started[h],
                        stop=final,
                    )
                    fold_started[h] = True

            # Software-pipelined two chunks ahead (3 PSUM S-slots).
            s_tiles = {0: emit_qk(0), 1: emit_qk(1)}
            e_tiles = {0: emit_e(0, s_tiles[0]), 1: emit_e(1, s_tiles[1])}
            for c in range(NCHUNK):
                if c + 2 < NCHUNK:
                    s_tiles[c + 2] = emit_qk(c + 2)
                    e_tiles[c + 2] = emit_e(c + 2, s_tiles[c + 2])
                s_tiles.pop(c)
                e16 = e_tiles.pop(c)

                if c == NCHUNK - 1:
                    # rs2 regions are complete; fold even, then e63 direct,
                    # all before the final AV pair so the reciprocal can
                    # overlap the PE drain.
                    emit_fold(rs2[:, 0:HW], final=False)
                    emit_fold(e16, final=True)

                for h in range(2):
                    nc.tensor.matmul(
                        o_ps[h][:],
                        vchunk(c),
                        e16[:, h * 512 : (h + 1) * 512],
                        start=(c == 0),
                        stop=(c == NCHUNK - 1),
                    )

                # Denominator partials (last chunk folds directly).
                if c < NCHUNK - 1:
                    reg = (c % 2) * HW
                    region = rs2[:, reg : reg + HW]
                    if c < 2:
                        nc.vector.tensor_copy(region, e16)
                    elif c in POOL_RS:
                        nc.gpsimd.tensor_add(region, region, e16)
                    else:
                        nc.vector.tensor_add(region, region, e16)

                if c == NCHUNK - 2:
                    # chunk 61 was the last odd-region contributor; fold it
                    # behind AV(62) on the PE.
                    rs_bc_ps = spsum.tile([D, HW], F32, tag="s", name="rs_bc_ps")
                    emit_fold(rs2[:, HW : 2 * HW], final=False)

            # --- endgame: rs_bc_ps holds the full denominator replicated
            # across partitions. Pipelined 4-slice normalize+store:
            # DVE: recip01, mult q0, recip23, mult q2;  Pool: mult q1, q3.
            rec_sb = [
                persist.tile([D, 512], F32, tag=f"rec{i}", name=f"rec{i}")
                for i in range(2)
            ]
            o_sb = [
                persist.tile([D, 256], F32, tag=f"osb{q}", name=f"osb{q}")
                for q in range(4)
            ]

            # GPSIMD can't read PSUM: slices q1/q3 get an ACT copy of the
            # O accumulator into SBUF, then a Pool multiply; q0/q2 multiply
            # straight from PSUM on DVE between the two reciprocals.
            o_cp = [
                persist.tile([D, 256], F32, tag=f"ocp{q}", name=f"ocp{q}")
                for q in range(2)
            ]

            def emit_mult(q, eng, src):
                h = q // 2
                eng.tensor_tensor(
                    o_sb[q][:],
                    src,
                    rec_sb[h][:, 256:512],
                    AluOpType.mult,
                )
                nc.sync.dma_start(out=oT[:, q * 256 : (q + 1) * 256], in_=o_sb[q][:])

            def emit_mult_psum(q):
                h = q // 2
                nc.vector.tensor_tensor(
                    o_sb[q][:],
                    o_ps[h][:, 0:256],
                    rec_sb[h][:, 0:256],
                    AluOpType.mult,
                )
                nc.sync.dma_start(out=oT[:, q * 256 : (q + 1) * 256], in_=o_sb[q][:])

            nc.scalar.activation(
                o_cp[0][:], o_ps[0][:, 256:512], mybir.ActivationFunctionType.Copy
            )
            nc.scalar.activation(
                o_cp[1][:], o_ps[1][:, 256:512], mybir.ActivationFunctionType.Copy
            )
            nc.vector.reciprocal(rec_sb[0][:], rs_bc_ps[:, 0:512])
            emit_mult_psum(0)
            emit_mult(1, nc.gpsimd, o_cp[0][:])
            nc.vector.reciprocal(rec_sb[1][:], rs_bc_ps[:, 512:1024])
            emit_mult_psum(2)
            emit_mult(3, nc.gpsimd, o_cp[1][:])

    nc.compile()
    return nc


_NC_CACHE = None


def _get_nc():
    global _NC_CACHE
    if _NC_CACHE is None:
        _NC_CACHE = _build_nc()
    return _NC_CACHE


def _prep_inputs(x: np.ndarray) -> list[dict]:
    x = np.ascontiguousarray(x, dtype=np.float32)
    xr = x.reshape(B, C, HW)

    # K channel-major over all tokens: kT[d, b*1024+hw] = x[b, 128+d, hw]
    kT = np.ascontiguousarray(
        xr[:, 128:256, :].transpose(1, 0, 2)
    ).reshape(D, N).astype(ml_dtypes.bfloat16)
    kT00 = np.ascontiguousarray(kT[:, 0:128])
    # V chunk-transposed fp16: vt[p, 128*j + v] = V[128*j + p, v]
    v_tok = np.ascontiguousarray(xr[:, 0:128, :].transpose(0, 2, 1)).reshape(N, D)
    vt16 = np.ascontiguousarray(
        v_tok.reshape(NCHUNK, 128, D).transpose(1, 0, 2)
    ).reshape(D, N).astype(np.float16)

    in_maps = []
    for c in range(N_CORES):
        qT = np.ascontiguousarray(xr[c, 256:384, :]).astype(ml_dtypes.bfloat16)
        in_maps.append({"qT": qT, "kT00": kT00, "kT": kT, "vt16": vt16})
    return in_maps


def kernel(x: np.ndarray) -> np.ndarray:
    assert x.shape == (B, C, H, W), x.shape
    in_maps = _prep_inputs(x)
    nc = _get_nc()
    res = run_bass_kernel_spmd(nc, in_maps, list(range(N_CORES)))

    out = np.empty((B, D, H, W), dtype=np.float32)
    for c in range(N_CORES):
        out[c] = res.results[c]["oT"].reshape(D, H, W)
    return out
